# revision 61
# baseline (speedup 1.0000x reference)
"""GCN (2x GCNConv + mean-pool + linear) on 8 Trainium2 cores, v3.

Key idea: GCNConv is linear up to the activation, so aggregate FIRST, then
GEMM:  A_norm @ (X @ W) == (A_norm @ X) @ W.  This removes every big dense
table materialization:

  - Layer 1: each core gathers raw xs rows (xs = dinv * x, staged by the host
    in DRAM as two 25088-row halves for int16 gather indices) for its own dst
    shard's in-edges, degree-chunk-major, accumulating in PSUM via
    identity-matmul.  Two gather "systems" (one per table half) with
    per-system degree-sorted slot orderings; system B is merged into system
    A's order with one HBM bounce + permutation gather + DVE add.  Then per
    128-token stripe: scale by dinv[dst] (DVE), PE-transpose, GEMM W1,
    ReLU+b1 -> feature-major h1T.  NO collective, NO replicated GEMM.
  - Layer 2: ys2 = dinv_src * (h1 @ W2) written to a 6272-row DRAM table;
    each core scatter-side aggregates for ALL 50k dst nodes (chunk-major over
    per-half degree-sorted slots), flushes PSUM chunks and dma_scatter_adds
    them into a host-zero-staged [25088, 256] accumulator (row r packs nodes
    r and r+25088; elem_step=256 keeps scatter indices int16).  One bf16
    ReduceScatter gives each core a 3136-row pair shard; node-major epilogue
    (dinv, +b2, relu) and one-hot mean-pool matmuls follow.
  - Host: sum the 8 [128, 256] pool partials, divide by counts, @ Wl + bl.
"""

import os
import numpy as np

DBG_STAGE = int(os.environ.get("KDBG_STAGE", "9"))  # 1=L1agg 2=+merge 3=+gemm/h1T 4=+ys2 5=+L2agg 6=+RS 9=all

N_NODES = 50000
N_EDGES = 800000
D = 128
N_GRAPHS = 256
CORES = 8
SHR = 6250               # real nodes per core shard
SH = 6272                # padded shard (49 * 128)
KCH = SH // 128          # 49
PAD = CORES * SH         # 50176
HALF = PAD // 2          # 25088
CH = 512                 # psum accumulation chunk width
PADG = 300.0
ZLOC = 6250              # a guaranteed-zero padded-local row in each half


def _pid(g):
    """global node id -> padded id (core*SH + local)."""
    return (g // SHR) * SH + (g % SHR)


def _wrap_idx(idx):
    idx = np.asarray(idx, np.int16)
    n = len(idx)
    assert n % 16 == 0
    base = idx.reshape(-1, 16).T
    return np.ascontiguousarray(np.tile(base, (8, 1)))


def _chunks(n):
    return [(c0, min(c0 + CH, n)) for c0 in range(0, n, CH)]


def _host_prep(x, edge_index, batch):
    x = np.asarray(x, np.float32)
    src = np.concatenate([edge_index[0], np.arange(N_NODES, dtype=np.int64)])
    dst = np.concatenate([edge_index[1], np.arange(N_NODES, dtype=np.int64)])
    deg = np.bincount(dst, minlength=N_NODES).astype(np.float32)
    dinv = (1.0 / np.sqrt(np.maximum(deg, 1.0))).astype(np.float32)

    # xs table, padded layout, two halves
    xs = np.zeros((PAD, D), np.float32)
    xs[_pid(np.arange(N_NODES))] = x * dinv[:, None]

    core_of_dst = dst // SHR
    core_of_src = src // SHR
    sp_id = _pid(src)
    sys_of_src = (sp_id >= HALF).astype(np.int64)

    # ---- per-core L1 structures (self-loops excluded: injected via a
    #      host-permuted xs table loaded with plain DMA instead) ----
    nonself = src != dst
    L1 = []
    for c in range(CORES):
        m = (core_of_dst == c) & nonself
        ltok = (dst[m] % SHR)                  # 0..6249
        s_ = sys_of_src[m]
        srow = sp_id[m] - s_ * HALF            # row within half table
        ent = {}
        for s in (0, 1):
            sel = s_ == s
            lt, sr = ltok[sel], srow[sel]
            d_s = np.bincount(lt, minlength=SH)
            order = np.argsort(-d_s, kind="stable")
            rank = np.empty(SH, np.int64)
            rank[order] = np.arange(SH)
            o2 = np.argsort(lt, kind="stable")
            ent[s] = dict(d=d_s, order=order, rank=rank,
                          srows=sr[o2], starts=np.searchsorted(lt[o2], np.arange(SH)),
                          nedge=len(lt))
        L1.append(ent)

    # SPMD shape: rounds per (sys, chunk) = max over cores; per-round widths
    # trimmed to ceil128(max active count) -- active sets are prefixes of the
    # degree-sorted chunk (round 0 always full width).
    ch1 = _chunks(SH)
    R1 = np.zeros((2, len(ch1)), np.int64)
    WL1 = {}
    for s in (0, 1):
        for k, (c0, c1) in enumerate(ch1):
            w = c1 - c0
            R = max(int(L1[c][s]["d"][L1[c][s]["order"][c0:c1]].max())
                    for c in range(CORES))
            R1[s, k] = R
            ws = []
            for j in range(R):
                na = max(int((L1[c][s]["d"][L1[c][s]["order"][c0:c1]] > j).sum())
                         for c in range(CORES))
                ws.append(w if j == 0 else min(w, -(-na // 128) * 128))
            WL1[(s, k)] = ws

    # ---- per-core L2 structures ----
    L2 = []
    for c in range(CORES):
        m = core_of_src == c
        dg = dst[m]                            # global dst 0..49999
        st_a = src[m] % SHR                    # local id of src
        ldeg = np.bincount(dg, minlength=HALF * 2)   # includes node ids up to 49999
        ent = {"ldeg": ldeg, "src_local": st_a, "dst": dg}
        for h in (0, 1):
            ids = np.arange(HALF) + h * HALF   # node ids of this half (incl >=N pad)
            dh = ldeg[ids]
            order = np.argsort(-dh, kind="stable")   # positions within half
            o2 = np.argsort(dg, kind="stable")
            ent[h] = dict(d=dh, order=order)
        ent["e_dst_sorted"] = np.argsort(dg, kind="stable")
        ent["starts"] = np.searchsorted(dg[ent["e_dst_sorted"]], np.arange(HALF * 2))
        L2.append(ent)

    ch2 = _chunks(HALF)                        # 49 chunks of 512
    R2 = np.zeros((2, len(ch2)), np.int64)
    NA2 = np.zeros((2, len(ch2)), np.int64)    # flush/scatter width (mult of 128)
    WL2 = {}
    for h in (0, 1):
        for k, (c0, c1) in enumerate(ch2):
            w = c1 - c0
            R = max(int(L2[c][h]["d"][L2[c][h]["order"][c0:c1]].max())
                    for c in range(CORES))
            R2[h, k] = R
            ws = []
            for j in range(R):
                na = max(int((L2[c][h]["d"][L2[c][h]["order"][c0:c1]] > j).sum())
                         for c in range(CORES))
                ws.append(w if j == 0 else min(w, -(-na // 128) * 128))
            WL2[(h, k)] = ws
            na = max(int((L2[c][h]["d"][L2[c][h]["order"][c0:c1]] > 0).sum())
                     for c in range(CORES))
            NA2[h, k] = -(-na // 128) * 128 if na else 0

    # ---- per-core input arrays ----
    cnts = np.bincount(np.asarray(batch, np.int64), minlength=N_GRAPHS).astype(np.float32)
    batch_full = np.asarray(batch, np.int64)
    core_inputs = []
    for c in range(CORES):
        # L1 gather idx streams (per system)
        idx1s = {0: [], 1: []}
        for s in (0, 1):
            e = L1[c][s]
            for k, (c0, c1) in enumerate(ch1):
                toks = e["order"][c0:c1]
                dloc = e["d"][toks]
                for j in range(int(R1[s, k])):
                    wj = WL1[(s, k)][j]
                    live = dloc[:wj] > j
                    epos = np.minimum(e["starts"][toks[:wj]] + j, max(e["nedge"] - 1, 0))
                    if e["nedge"]:
                        r = np.where(live, e["srows"][epos], ZLOC)
                    else:
                        r = np.full(wj, ZLOC, np.int64)
                    idx1s[s].append(r)
        idx1 = _wrap_idx(np.concatenate(idx1s[0]) if idx1s[0] else np.zeros(0, np.int64))
        idx1b = _wrap_idx(np.concatenate(idx1s[1]) if idx1s[1] else np.zeros(0, np.int64))

        # self-loop init tables: xsp[s][slot] = xs[node at slot] for the
        # self system (src half containing this core), zeros for the other
        s_self = 1 if c >= CORES // 2 else 0
        xsp = {}
        for s in (0, 1):
            t = np.zeros((SH, D), np.float32)
            if s == s_self:
                osl = L1[c][s]["order"]
                real = osl < SHR
                gid = c * SHR + np.minimum(osl, SHR - 1)
                t[real] = xs[_pid(gid)][real]
            xsp[s] = t
        xsp0, xsp1 = xsp[0], xsp[1]

        # merge permutation: accB row for orderA slot t
        oA, rB = L1[c][0]["order"], L1[c][1]["rank"]
        permB = _wrap_idx(rB[oA])

        # dinvA: dinv of node at orderA slot (p,k); 0 for pads
        gl = c * SHR + np.minimum(oA, SHR - 1)
        dva = np.where(oA < SHR, dinv[np.minimum(gl, N_NODES - 1)], 0.0).astype(np.float32)
        dinvA = np.ascontiguousarray(dva.reshape(KCH, 128).T)

        # L2 gather idx stream (src tokens in orderA numbering) + scatter idx
        rA = L1[c][0]["rank"]
        e2 = L2[c]
        src_tok = rA[e2["src_local"]][e2["e_dst_sorted"]]  # edge j of dst n at starts[n]+j
        ZROW2 = int(rA[ZLOC])
        idx2, idxsc = [], []
        for h in (0, 1):
            for k, (c0, c1) in enumerate(ch2):
                if R2[h, k] == 0:
                    continue
                ids = (e2[h]["order"][c0:c1]) + h * HALF    # node ids in slot order
                dloc = e2["ldeg"][ids]
                for j in range(int(R2[h, k])):
                    wj = WL2[(h, k)][j]
                    live = dloc[:wj] > j
                    epos = np.minimum(e2["starts"][ids[:wj]] + j,
                                      len(src_tok) - 1 if len(src_tok) else 0)
                    if len(src_tok):
                        r = np.where(live, src_tok[epos], ZROW2)
                    else:
                        r = np.full(wj, ZROW2, np.int64)
                    idx2.append(r)
                na = int(NA2[h, k])
                if na:
                    idxsc.append(ids[:na] % HALF)   # row index (pairing), all real adds
        idx2 = _wrap_idx(np.concatenate(idx2) if idx2 else np.zeros(0, np.int64))
        idxsc = _wrap_idx(np.concatenate(idxsc) if idxsc else np.zeros(0, np.int64))

        # shard constants: token (p,k) -> node n = 3136c + b//2 + (b%2)*HALF, b=p*49+k
        b = (np.arange(128)[:, None] * KCH + np.arange(KCH)[None, :])
        r_ = 3136 * c + b // 2
        n_ = r_ + (b % 2) * HALF
        valid = n_ < N_NODES
        nv = np.minimum(n_, N_NODES - 1)
        dinv2 = np.where(valid, dinv[nv], 0.0).astype(np.float32)
        batchR2 = np.where(valid, batch_full[nv].astype(np.float64), PADG).astype(np.float32)

        core_inputs.append(dict(idx1=idx1, idx1b=idx1b, permB=permB, dinvA=dinvA,
                                xsp0=xsp0, xsp1=xsp1,
                                idx2=idx2, idxsc=idxsc,
                                dinv2=np.ascontiguousarray(dinv2),
                                batchR2=np.ascontiguousarray(batchR2)))

    plan = dict(R1=R1, R2=R2, NA2=NA2, WL1=WL1, WL2=WL2,
                W1=idx1.shape[1], W1b=idx1b.shape[1],
                W2w=idx2.shape[1], Wsc=idxsc.shape[1])
    # idx widths identical across cores by construction (SPMD max shapes)
    for ci in core_inputs:
        assert ci["idx1"].shape[1] == plan["W1"]
        assert ci["idx1b"].shape[1] == plan["W1b"]
        assert ci["idx2"].shape[1] == plan["W2w"]
        assert ci["idxsc"].shape[1] == plan["Wsc"]
    return xs, core_inputs, plan, cnts


def _build(plan):
    import concourse.mybir as mybir
    import concourse.tile as tile
    from concourse import bacc, library_config

    f32 = mybir.dt.float32
    bf16 = mybir.dt.bfloat16
    i16 = mybir.dt.int16
    Alu = mybir.AluOpType
    Act = mybir.ActivationFunctionType
    R1, R2, NA2 = plan["R1"], plan["R2"], plan["NA2"]
    WL1, WL2 = plan["WL1"], plan["WL2"]
    W1w, W1bw, W2w, Wsc = plan["W1"], plan["W1b"], plan["W2w"], plan["Wsc"]
    ch1 = _chunks(SH)
    ch2 = _chunks(HALF)

    nc = bacc.Bacc(None, target_bir_lowering=False, num_devices=CORES)
    with tile.TileContext(nc) as tc:
        with tc.tile_pool(name="dram", bufs=1, space="DRAM") as dram, \
             tc.tile_pool(name="cst", bufs=1) as cst, \
             tc.tile_pool(name="big", bufs=1) as big, \
             tc.tile_pool(name="gt", bufs=6) as gt, \
             tc.tile_pool(name="stg", bufs=5) as stg, \
             tc.tile_pool(name="pa", bufs=3, space="PSUM") as pa, \
             tc.tile_pool(name="pt", bufs=2, space="PSUM") as pt, \
             tc.tile_pool(name="pg", bufs=2, space="PSUM") as pg, \
             tc.tile_pool(name="pp", bufs=1, space="PSUM") as pp:

            # ---------------- I/O ----------------
            xs_lo_d = dram.tile((HALF, D), bf16, kind="ExternalInput")
            xs_hi_d = dram.tile((HALF, D), bf16, kind="ExternalInput")
            xsp0_d = dram.tile((SH, D), bf16, kind="ExternalInput")
            xsp1_d = dram.tile((SH, D), bf16, kind="ExternalInput")
            W1_d = dram.tile((D, D), bf16, kind="ExternalInput")
            W2_d = dram.tile((D, D), bf16, kind="ExternalInput")
            b1_d = dram.tile((D, 1), f32, kind="ExternalInput")
            c2b_d = dram.tile((D, KCH, D), f32, kind="ExternalInput")
            dinvA_d = dram.tile((D, KCH), f32, kind="ExternalInput")
            dinv2_d = dram.tile((D, KCH), f32, kind="ExternalInput")
            batchR2_d = dram.tile((D, KCH), f32, kind="ExternalInput")
            iota_d = dram.tile((D, N_GRAPHS), f32, kind="ExternalInput")
            identb_d = dram.tile((D, D), bf16, kind="ExternalInput")
            idx1_d = dram.tile((128, W1w), i16, kind="ExternalInput")
            idx1b_d = dram.tile((128, W1bw), i16, kind="ExternalInput")
            idx2_d = dram.tile((128, W2w), i16, kind="ExternalInput")
            idxsc_d = dram.tile((128, Wsc), i16, kind="ExternalInput")
            permB_d = dram.tile((128, SH // 16), i16, kind="ExternalInput")
            acc_d = dram.tile((HALF, 256), bf16)
            pool_d = dram.tile((D, N_GRAPHS), f32, kind="ExternalOutput")

            accB_d = dram.tile((SH, D), bf16)
            ys2_d = dram.tile((SH, D), bf16)
            shard_d = dram.tile((128, KCH, 128), bf16)

            nc.gpsimd.load_library(library_config.mlp)

            # ---------------- SBUF ----------------
            W1s = cst.tile([D, D], bf16)
            W2s = cst.tile([D, D], bf16)
            b1s = cst.tile([D, 1], f32)
            c2b = cst.tile([D, KCH, D], f32)
            dinvA = cst.tile([D, KCH], f32)
            dinv2 = cst.tile([D, KCH], f32)
            batchR2 = cst.tile([D, KCH], f32)
            iota = cst.tile([D, N_GRAPHS], f32)
            identb = cst.tile([D, D], bf16)
            idx1 = cst.tile([128, W1w], i16)
            idx1b = cst.tile([128, W1bw], i16)
            idx2 = cst.tile([128, W2w], i16)
            idxsc = cst.tile([128, Wsc], i16)
            permB = cst.tile([128, SH // 16], i16)

            h1T = big.tile([128, SH], bf16)
            xsp0s = big.tile([128, KCH, 128], bf16)
            xsp1s = big.tile([128, KCH, 128], bf16)
            shsb = big.tile([128, KCH, 128], bf16)

            hb = max(16, (W1bw // 16) & ~15)
            nc.sync.dma_start(out=idx1b[:, :hb], in_=idx1b_d[:, :hb])
            nc.sync.dma_start(out=identb[:], in_=identb_d[:])
            nc.sync.dma_start(out=idx1b[:, hb:], in_=idx1b_d[:, hb:])
            nc.sync.dma_start(out=xsp1s[:],
                              in_=xsp1_d[:].rearrange("(a p) f -> p a f", p=128))
            nc.sync.dma_start(out=xsp0s[:],
                              in_=xsp0_d[:].rearrange("(a p) f -> p a f", p=128))
            for sb, d in [(W1s, W1_d), (W2s, W2_d), (b1s, b1_d), (c2b, c2b_d),
                          (dinvA, dinvA_d), (dinv2, dinv2_d), (batchR2, batchR2_d),
                          (iota, iota_d), (idx1, idx1_d),
                          (idx2, idx2_d), (idxsc, idxsc_d), (permB, permB_d)]:
                nc.sync.dma_start(out=sb[:], in_=d[:])

            # zero the RS input accumulator (internal DRAM) via broadcast
            # DMA, split into pieces so no single long transfer blocks the
            # DMA pipeline behind it
            zt = cst.tile([128, 256], bf16)
            nc.vector.memset(zt[:], 0.0)
            NZ = HALF // 128 // 12
            for z0 in range(0, HALF // 128, NZ):
                zn = min(NZ, HALF // 128 - z0)
                nc.sync.dma_start(
                    out=acc_d[z0 * 128:(z0 + zn) * 128, :]
                        .rearrange("(a p) f -> p a f", p=128),
                    in_=zt[:].rearrange("p (a f) -> p a f", a=1).broadcast_to(
                        [128, zn, 256]))

            dbg = {}

            # ------------- L1 system B (order-B slots) -> accB_d -------------
            off = [0]   # running column offset into the active idx tensor

            def agg_chunk(xs_d, xsp_d, c0, c1, ws, idx, stop_last=True):
                # psum initialized by a full-width matmul of the host-permuted
                # self-loop table (plain DMA, off the Pool engine); gather
                # rounds then accumulate with trimmed prefix widths.
                w = c1 - c0
                R = len(ws)
                ps_full = pa.tile([128, CH], f32, space="PSUM", tag="pa")
                ps = ps_full[:, :w]
                j = 0
                while j < R:
                    jn = 1
                    tot = ws[j]
                    while j + jn < R and tot + ws[j + jn] <= 1024:
                        tot += ws[j + jn]
                        jn += 1
                    g = gt.tile([128, 8, 128], bf16, tag="g1")
                    nc.gpsimd.dma_gather(g[:, :tot // 128, :], xs_d[:],
                                         idx[:, off[0]:off[0] + tot // 16],
                                         tot, tot, D)
                    goff = 0
                    for u in range(jn):
                        wj = ws[j + u]
                        nc.tensor.matmul(
                            ps_full[:, :wj], lhsT=identb[:],
                            rhs=g[:, goff // 128:(goff + wj) // 128, :]
                                .rearrange("p a b -> p (a b)"),
                            start=(j + u == 0), stop=False)
                        goff += wj
                    off[0] += tot // 16
                    j += jn
                nc.tensor.matmul(
                    ps, lhsT=identb[:],
                    rhs=xsp_d[:, c0 // 128:c1 // 128, :].rearrange("p a b -> p (a b)"),
                    start=(R == 0), stop=stop_last)
                return ps

            for k, (c0, c1) in enumerate(ch1):
                w = c1 - c0
                ps = agg_chunk(xs_hi_d, xsp1s, c0, c1, WL1[(1, k)], idx1b)
                bt = stg.tile([128, CH // 128, 128], bf16, tag="bt")
                nc.vector.tensor_copy(
                    out=bt[:, :w // 128, :],
                    in_=ps.rearrange("p (a b) -> p a b", b=128))
                nc.scalar.dma_start(
                    out=accB_d[c0:c1, :].rearrange("(a p) f -> p a f", p=128),
                    in_=bt[:, :w // 128, :])

            # --- L1 system A, fused: merge + GEMM1 + epilogue + GEMM2 + ys2 ---
            # Chunks processed in REVERSE degree order: the largest chunk runs
            # last, so its long gather stream covers the DVE/ACT drain of all
            # trailing merge/GEMM/ys2 work before L2 needs ys2_d complete.
            # (merge/GEMM of the previous chunk is emitted after the current
            #  chunk's gathers so perm-gathers don't stall the Pool queue.)
            off1c = np.concatenate([[0], np.cumsum(
                [sum(WL1[(0, k)]) // 16 for k in range(len(ch1))])])

            def sysa_chunk(k, c0, c1):
                w = c1 - c0
                kw = w // 128
                R = int(R1[0, k])
                off[0] = int(off1c[k])
                ps = agg_chunk(xs_lo_d, xsp0s, c0, c1, WL1[(0, k)], idx1,
                               stop_last=False)
                # fold the permuted system-B contribution into the same psum
                gB = gt.tile([128, CH // 128, 128], bf16, tag="gB")
                nc.gpsimd.dma_gather(gB[:, :kw, :], accB_d[:],
                                     permB[:, c0 // 16:c1 // 16], w, w, D)
                nc.tensor.matmul(ps, lhsT=identb[:],
                                 rhs=gB[:, :kw, :].rearrange("p a b -> p (a b)"),
                                 start=False, stop=True)
                am = stg.tile([128, CH // 128, 128], bf16, tag="am")
                if k % 2 == 0:
                    nc.vector.tensor_copy(out=am[:, :kw, :],
                                          in_=ps.rearrange("p (a b) -> p a b", b=128))
                else:
                    nc.scalar.activation(out=am[:, :kw, :],
                                         in_=ps.rearrange("p (a b) -> p a b", b=128),
                                         func=Act.Copy, bias=0.0, scale=1.0)
                # scale by dinv[dst], 4 transposes into one psum, then
                # 512-wide GEMM1+ReLU
                pt4 = pt.tile([128, CH], bf16, space="PSUM", tag="pt1")
                for q in range(kw):
                    kk = c0 // 128 + q
                    st = stg.tile([128, 128], bf16, tag="st1")
                    if q % 2 == 0:
                        nc.vector.tensor_scalar(out=st[:], in0=am[:, q, :],
                                                scalar1=dinvA[:, kk:kk + 1], scalar2=None,
                                                op0=Alu.mult)
                    else:
                        nc.scalar.activation(out=st[:], in_=am[:, q, :], func=Act.Copy,
                                             bias=0.0, scale=dinvA[:, kk:kk + 1])
                    nc.tensor.transpose(pt4[:, q * 128:(q + 1) * 128], st[:], identb[:])
                at = stg.tile([128, CH // 128, 128], bf16, tag="at1")
                if k % 2 == 0:
                    nc.scalar.activation(out=at[:, :kw, :],
                                         in_=pt4[:, :w].rearrange("p (a b) -> p a b", b=128),
                                         func=Act.Copy, bias=0.0, scale=1.0)
                else:
                    nc.vector.tensor_copy(out=at[:, :kw, :],
                                          in_=pt4[:, :w].rearrange("p (a b) -> p a b", b=128))
                ps2 = pg.tile([128, CH], f32, space="PSUM", tag="pg1")
                nc.tensor.matmul(ps2[:, :w], lhsT=W1s[:],
                                 rhs=at[:, :kw, :].rearrange("p a b -> p (a b)"),
                                 start=True, stop=True)
                nc.scalar.activation(out=h1T[:, c0:c1], in_=ps2[:, :w],
                                     func=Act.Relu, bias=b1s[:, :1], scale=1.0)
                ps3 = pg.tile([128, CH], f32, space="PSUM", tag="pg1")
                for q in range(kw):
                    kk = c0 // 128 + q
                    nc.tensor.matmul(ps3[:, q * 128:(q + 1) * 128],
                                     lhsT=h1T[:, kk * 128:(kk + 1) * 128],
                                     rhs=W2s[:], start=True, stop=True)
                ysb = stg.tile([128, CH // 128, 128], bf16, tag="ysb")
                for q in range(kw):
                    kk = c0 // 128 + q
                    if q % 2 == 0:
                        nc.vector.tensor_scalar(out=ysb[:, q, :],
                                                in0=ps3[:, q * 128:(q + 1) * 128],
                                                scalar1=dinvA[:, kk:kk + 1], scalar2=None,
                                                op0=Alu.mult)
                    else:
                        nc.scalar.activation(out=ysb[:, q, :],
                                             in_=ps3[:, q * 128:(q + 1) * 128],
                                             func=Act.Copy, bias=0.0,
                                             scale=dinvA[:, kk:kk + 1])
                nc.sync.dma_start(
                    out=ys2_d[c0:c1, :].rearrange("(a p) f -> p a f", p=128),
                    in_=ysb[:, :kw, :])

            for k in range(len(ch1)):
                sysa_chunk(k, *ch1[k])

            if DBG_STAGE == 3:
                dbg_out = dram.tile((128, SH), bf16, kind="ExternalOutput")
                nc.sync.dma_start(out=dbg_out[:], in_=h1T[:])
                dbg["dbg"] = dbg_out.name

            # ------------- L2 aggregation + scatter -------------
            off2 = 0
            offsc = 0
            for h in ((0, 1) if DBG_STAGE >= 5 else ()):
                for k in range(len(ch2)):
                    R = int(R2[h, k])
                    if R == 0:
                        continue
                    ws = WL2[(h, k)]
                    ps = pa.tile([128, CH], f32, space="PSUM", tag="pa")
                    j = 0
                    while j < R:
                        jn = 1
                        tot = ws[j]
                        while j + jn < R and tot + ws[j + jn] <= 1024:
                            tot += ws[j + jn]
                            jn += 1
                        g = gt.tile([128, 8, 128], bf16, tag="g2")
                        nc.gpsimd.dma_gather(g[:, :tot // 128, :], ys2_d[:],
                                             idx2[:, off2:off2 + tot // 16],
                                             tot, tot, D)
                        goff = 0
                        for u in range(jn):
                            wj = ws[j + u]
                            nc.tensor.matmul(ps[:, :wj], lhsT=identb[:],
                                             rhs=g[:, goff // 128:(goff + wj) // 128, :]
                                                 .rearrange("p a b -> p (a b)"),
                                             start=(j + u == 0), stop=(j + u == R - 1))
                            goff += wj
                        off2 += tot // 16
                        j += jn
                    na = int(NA2[h, k])
                    if na == 0:
                        continue
                    stf = stg.tile([128, na // 128, 128], bf16, tag="stf")
                    if k % 2 == 0:
                        nc.vector.tensor_copy(
                            out=stf[:],
                            in_=ps[:, :na].rearrange("p (a b) -> p a b", b=128))
                    else:
                        nc.scalar.activation(
                            out=stf[:],
                            in_=ps[:, :na].rearrange("p (a b) -> p a b", b=128),
                            func=Act.Copy, bias=0.0, scale=1.0)
                    nc.gpsimd.dma_scatter_add(
                        acc_d[:, h * 128:(h + 1) * 128], stf[:],
                        idxsc[:, offsc:offsc + na // 16], na, na, D, elem_step=256)
                    offsc += na // 16

            if DBG_STAGE == 5:
                dbg_out = dram.tile((128, HALF * 2), bf16, kind="ExternalOutput")
                nc.sync.dma_start(
                    out=dbg_out[:].rearrange("p (a f) -> p a f", f=256),
                    in_=acc_d[:].rearrange("(a p) f -> p a f", p=128))
                dbg["dbg"] = dbg_out.name

            # ------------- ReduceScatter + epilogue + pool -------------
            # G one-hot masks only depend on constants: build them before the
            # RS so the work hides under the L2 gather stream.
            Gbig = big.tile([128, KCH, N_GRAPHS], bf16)
            for k in range(KCH):
                nc.vector.tensor_scalar(out=Gbig[:, k, :], in0=iota[:],
                                        scalar1=batchR2[:, k:k + 1], scalar2=None,
                                        op0=Alu.is_equal)
            if DBG_STAGE >= 6:
                nc.gpsimd.collective_compute(
                    "ReduceScatter", Alu.add, replica_groups=[list(range(CORES))],
                    ins=[acc_d.opt()], outs=[shard_d.opt()])
            else:
                nc.vector.memset(shsb[:], 0.0)
            pool_ps = pp.tile([128, N_GRAPHS], f32, space="PSUM")
            # dinv2 and +b2 folded as relu((x + b2/d) * d): c2 = b2/d staged
            # as a constant; one wide DVE add + one wide ACT scale-relu.
            for k0 in range(0, KCH, 4):
                kn = min(4, KCH - k0)
                g3 = (k0 // 4) % 3
                if DBG_STAGE >= 6:
                    nc.sync.dma_start(out=shsb[:, k0:k0 + kn, :],
                                      in_=shard_d[:, k0:k0 + kn, :])
                t1 = stg.tile([128, 4, 128], f32, tag="t1")
                adder = (nc.vector, nc.gpsimd, nc.vector)[g3]
                adder.tensor_tensor(out=t1[:, :kn, :], in0=shsb[:, k0:k0 + kn, :],
                                    in1=c2b[:, k0:k0 + kn, :], op=Alu.add)
                for q in range(kn):
                    k = k0 + q
                    h2 = stg.tile([128, 128], bf16, tag="h2")
                    if q % 2 == 0:
                        nc.scalar.activation(out=h2[:], in_=t1[:, q, :], func=Act.Relu,
                                             bias=0.0, scale=dinv2[:, k:k + 1])
                    else:
                        eng = (nc.gpsimd, nc.vector)[g3 % 2]
                        eng.tensor_scalar(out=h2[:], in0=t1[:, q, :],
                                          scalar1=dinv2[:, k:k + 1], scalar2=0.0,
                                          op0=Alu.mult, op1=Alu.max)
                    nc.tensor.matmul(pool_ps[:], lhsT=h2[:], rhs=Gbig[:, k, :],
                                     start=(k == 0), stop=(k == KCH - 1))
            outsb = stg.tile([128, N_GRAPHS], f32, tag="outsb")
            nc.vector.tensor_copy(out=outsb[:], in_=pool_ps[:])
            nc.sync.dma_start(out=pool_d[:], in_=outsb[:])

    nc.compile()
    names = dict(
        xs_lo=xs_lo_d.name, xs_hi=xs_hi_d.name,
        xsp0=xsp0_d.name, xsp1=xsp1_d.name, W1=W1_d.name, W2=W2_d.name,
        b1=b1_d.name, c2b=c2b_d.name, dinvA=dinvA_d.name, dinv2=dinv2_d.name,
        batchR2=batchR2_d.name, iota=iota_d.name, identb=identb_d.name,
        idx1=idx1_d.name, idx1b=idx1b_d.name, idx2=idx2_d.name, idxsc=idxsc_d.name, permB=permB_d.name,
        out=pool_d.name)
    names.update(dbg)
    return nc, names


_cache = {}
_last_in_maps = None


def kernel(x, edge_index, batch, W1, b1, W2, b2, Wl, bl):
    from concourse.bass_utils import run_bass_kernel_spmd
    import ml_dtypes
    bf = ml_dtypes.bfloat16

    x = np.asarray(x, np.float32)
    edge_index = np.asarray(edge_index)
    batch = np.asarray(batch)
    W1 = np.asarray(W1, np.float32); b1 = np.asarray(b1, np.float32)
    W2 = np.asarray(W2, np.float32); b2 = np.asarray(b2, np.float32)
    Wl = np.asarray(Wl, np.float32); bl = np.asarray(bl, np.float32)

    xs, core_inputs, plan, cnts = _host_prep(x, edge_index, batch)

    key = (tuple(plan["R1"].ravel()), tuple(plan["R2"].ravel()),
           tuple(plan["NA2"].ravel()),
           tuple(tuple(v) for _, v in sorted(plan["WL1"].items())),
           tuple(tuple(v) for _, v in sorted(plan["WL2"].items())))
    if key not in _cache:
        _cache[key] = _build(plan)
    nc, nm = _cache[key]

    iota_b = np.broadcast_to(np.arange(N_GRAPHS, dtype=np.float32)[None, :],
                             (D, N_GRAPHS)).copy()
    identb = np.eye(D, dtype=np.float32).astype(bf)
    xs_lo = xs[:HALF].astype(bf)
    xs_hi = xs[HALF:].astype(bf)
    in_maps = []
    for ci in core_inputs:
        in_maps.append({
            nm["xs_lo"]: xs_lo, nm["xs_hi"]: xs_hi,
            nm["xsp0"]: ci["xsp0"].astype(bf), nm["xsp1"]: ci["xsp1"].astype(bf),
            nm["W1"]: W1.astype(bf), nm["W2"]: W2.astype(bf),
            nm["b1"]: b1.reshape(D, 1),
            nm["c2b"]: np.where(ci["dinv2"].T[:, :, None] > 0,
                                b2[None, None, :] / np.maximum(ci["dinv2"].T[:, :, None], 1e-9),
                                0.0).astype(np.float32).transpose(1, 0, 2).copy(),
            nm["dinvA"]: ci["dinvA"], nm["dinv2"]: ci["dinv2"],
            nm["batchR2"]: ci["batchR2"], nm["iota"]: iota_b,
            nm["identb"]: identb,
            nm["idx1"]: ci["idx1"], nm["idx1b"]: ci["idx1b"], nm["idx2"]: ci["idx2"],
            nm["idxsc"]: ci["idxsc"], nm["permB"]: ci["permB"],
        })
    global _last_in_maps
    _last_in_maps = in_maps
    res = run_bass_kernel_spmd(nc, in_maps, list(range(CORES)))
    total = np.zeros((D, N_GRAPHS), np.float32)
    for r in res.results:
        total += r[nm["out"]]
    pooled = (total / np.maximum(cnts, 1.0)[None, :]).T
    return (pooled @ Wl + bl).astype(np.float32)
